# revision 15
# baseline (speedup 1.0000x reference)
"""Trainium2 Bass kernel for nn_AlphaEntmax (entmax-bisect over last axis).

Key math fact: the module's ClampMin/ClampMax composition maps any alpha in
[1,2] to exactly 2.0, so the reference computes sparsemax (alpha=2) per row:
    p = relu(x - tau) / sum(relu(x - tau)),  tau s.t. sum(relu(x - tau)) = 1
We solve for tau with 4 over-relaxed Newton steps from tau0 = rowmax - 0.8
(tau' = tau + lam*(r-1)/c with r = sum(relu(x-tau)), c = count(x > tau),
lam = [1.15, 1.10, 1.05, 1.0]), then emit p = relu(x - tau) directly (sum==1
at convergence; the reference's own normalize brings both within tolerance).

Engine split (per [128,1024] tile, 64 tiles per core):
  - r-passes need no relu op: r = Sx - sum(min(x, tau)) with Sx = sum(x)
    computed once, so each r-pass is a plain VectorE tensor_scalar(op0=min,
    accum=add) on the fp16 shadow in 4x DVE mode (~300ns). The min-sum's
    magnitude is ~|Sx| ~ 30 (the inactive elements keep their raw values,
    mean 0), so the hardware f32 accumulator drift is ~1e-5. The naive
    dual  sum(max(x,tau)) = K*tau + r  looked cheaper (no Sx pass) but its
    accumulated magnitude ~2500 costs ~5e-2 of drift on HW — unusable.
  - c-passes: VectorE is_gt (fp16 4x); one slot per group is staggered onto
    ScalarE via ACT Sign(scale=-1, bias=tau): acc = K - 2c; and slot 0 of
    one group per wave can go to GPSIMD (tensor_scalar is_gt).
  - fp16 shadow (not bf16): 8x lower quantization floor, same 4x speed.
  - final pass on ScalarE: ACT Relu(x_f32 + ntau), written in place over
    the f32 x tile, stored from the Pool SWDGE queue (loads use SP) so
    loads and stores never head-of-line block each other.
  - stats live in one [128, 9*12] tile per 12-tile wave: each per-slot
    update chain is ~6 small VectorE ops for the whole wave.
DMA is the roofline: 32MB in + 32MB out per core over ~360GB/s ≈ 186us.

Sharding: x [8,16,512,1024] is split along the batch axis, one batch entry
(8192 rows of 1024) per NeuronCore; no cross-core communication.
"""

import numpy as np

B, H, Q, K = 8, 16, 512, 1024
N_CORES = 8
P = 128
ROWS_PER_CORE = (B // N_CORES) * H * Q  # 8192
N_TILES = ROWS_PER_CORE // P  # 64
G = 4  # tiles per group (DMA/cast granularity)
N_GROUPS = N_TILES // G  # 16
N_SLOTS = 4
D0 = 0.8  # tau0 = rowmax - D0
LAM = [1.15, 1.10, 1.05, 1.0]  # per-slot Newton over-relaxation
W = 3  # groups per wave; stats + update chains are per-wave
# count-pass engine rotation: at slot i, group (i mod W) counts on ScalarE
# (ACT Sign), 3 of 4 tiles of group ((i+1) mod W) on GPSIMD, rest VectorE
POOL_CNT_TILES = 3  # tiles per Pool-assigned group that actually go to Pool
V_FINAL_KPOS = (0,)  # group positions whose final runs on VectorE (in-place)
BUFS = {"xp": 9, "hp": 6, "st": 4}

_NC_CACHE = None


def _build_nc():
    import concourse.bacc as bacc
    import concourse.mybir as mybir
    from concourse.tile import TileContext

    f32 = mybir.dt.float32
    f16 = mybir.dt.float16
    Alu = mybir.AluOpType
    Act = mybir.ActivationFunctionType

    nc = bacc.Bacc(
        "TRN2", target_bir_lowering=False, debug=False, num_devices=N_CORES
    )
    x_ext = nc.dram_tensor("x", [ROWS_PER_CORE, K], f32, kind="ExternalInput")
    out_ext = nc.dram_tensor("out", [ROWS_PER_CORE, K], f32, kind="ExternalOutput")

    waves = []
    g0 = 0
    while g0 < N_GROUPS:
        waves.append(list(range(g0, min(g0 + W, N_GROUPS))))
        g0 += W
    NW = len(waves)

    ST_NAMES = ("mx", "tau", "ntau", "sx", "acc", "cacc", "r", "c", "rcp", "stp")

    with TileContext(nc) as tc:
        with (
            tc.tile_pool(name="xp", bufs=BUFS["xp"]) as xp,
            tc.tile_pool(name="hp", bufs=BUFS["hp"]) as hp,
            tc.tile_pool(name="scr", bufs=1) as scr,
            tc.tile_pool(name="st", bufs=BUFS["st"]) as st,
        ):
            # engine-dedicated elementwise-output scratch (never read back)
            scrR = scr.tile([P, K], f16, tag="scrR")  # V r/Sx out
            scrC = scr.tile([P, K], f16, tag="scrC")  # V c-pass out
            scrS = scr.tile([P, K], f16, tag="scrS", name="scrS")  # S sign out
            scrP = scr.tile([P, K], f16, tag="scrP", name="scrP")  # Pool out

            # warm the ACT table (a set containing Sign AND Relu) so the
            # one-time ~2.7us table load overlaps the first DMA
            nc.scalar.activation(
                scrS[:, :1], nc.const_aps.aps[(f32, 0.0)], Act.Sign
            )
            nc.scalar.activation(
                scrS[:, 1:2], nc.const_aps.aps[(f32, 0.0)], Act.Relu
            )

            # ---- per-wave state ----
            xbs = {}   # g -> xb tile
            xhs = {}   # g -> xh tile
            wstt = {}  # w -> dict name -> AP over [P, n_cols(w)]
            wcols = {}  # w -> n tiles in wave

            def emit_wave_loads(w):
                for g in waves[w]:
                    rows = slice(g * G * P, (g + 1) * G * P)
                    x_dram = x_ext.ap()[rows, :].rearrange(
                        "(t p) k -> p t k", p=P
                    )
                    xb = xp.tile([P, G * K], f32, tag="xb")
                    nc.sync.dma_start(
                        out=xb[:].rearrange("p (t k) -> p t k", t=G), in_=x_dram
                    )
                    xbs[g] = xb

            def col(w, g, j):
                # stats column index for tile j of group g within wave w
                return waves[w].index(g) * G + j

            def alloc_wave_stats(w):
                ncols = len(waves[w]) * G
                wcols[w] = ncols
                st_t = st.tile([P, len(ST_NAMES) * ncols], f32, tag="st")
                wstt[w] = {
                    n: st_t[:, k * ncols : (k + 1) * ncols]
                    for k, n in enumerate(ST_NAMES)
                }

            def emit_cast_group(w, kpos):
                stt = wstt[w]
                g = waves[w][kpos]
                xh = hp.tile([P, G * K], f16, tag="xh")
                xhs[g] = xh
                xb = xbs[g]
                for j in range(G):
                    cidx = kpos * G + j
                    # fp16 shadow + exact row max via f32 accum (pre-cast)
                    nc.vector.tensor_scalar(
                        xh[:, j * K : (j + 1) * K],
                        xb[:, j * K : (j + 1) * K],
                        0.0, None, Alu.add, Alu.max,
                        accum_out=stt["mx"][:, cidx : cidx + 1],
                    )
                    # Sx = sum(xh)  (fp16 4x pass)
                    nc.vector.tensor_scalar(
                        scrR[:], xh[:, j * K : (j + 1) * K],
                        0.0, None, Alu.add, Alu.add,
                        accum_out=stt["sx"][:, cidx : cidx + 1],
                    )
                # tau0 = mx - D0
                sl = slice(kpos * G, (kpos + 1) * G)
                nc.vector.tensor_scalar(
                    stt["tau"][:, sl], stt["mx"][:, sl], -D0, None, Alu.add
                )

            def emit_wave_casts(w):
                alloc_wave_stats(w)
                for kpos in range(len(waves[w])):
                    emit_cast_group(w, kpos)

            def emit_group_mins(w, i, kpos):
                stt = wstt[w]
                g = waves[w][kpos]
                xh = xhs[g]
                for j in range(G):
                    cidx = kpos * G + j
                    # acc = sum(min(x, tau));  r = Sx - acc
                    nc.vector.tensor_scalar(
                        scrR[:], xh[:, j * K : (j + 1) * K],
                        stt["tau"][:, cidx : cidx + 1], None,
                        Alu.min, Alu.add,
                        accum_out=stt["acc"][:, cidx : cidx + 1],
                    )

            def emit_group_counts(w, i, kpos, eng):
                stt = wstt[w]
                g = waves[w][kpos]
                xh, xb = xhs[g], xbs[g]
                for j in range(G):
                    cidx = kpos * G + j
                    tau_j = stt["tau"][:, cidx : cidx + 1]
                    xhj = xh[:, j * K : (j + 1) * K]
                    if eng == "S":
                        # cacc = sum(sign(tau - x)) = K - 2c
                        nc.scalar.activation(
                            scrS[:], xb[:, j * K : (j + 1) * K], Act.Sign,
                            bias=tau_j, scale=-1.0,
                            accum_out=stt["cacc"][:, cidx : cidx + 1],
                        )
                    elif eng == "P" and j < POOL_CNT_TILES:
                        nc.gpsimd.tensor_scalar(
                            scrP[:], xhj, tau_j, None,
                            Alu.is_gt, Alu.add,
                            accum_out=stt["cacc"][:, cidx : cidx + 1],
                        )
                    else:
                        nc.vector.tensor_scalar(
                            scrC[:], xhj, tau_j, None,
                            Alu.is_gt, Alu.add,
                            accum_out=stt["cacc"][:, cidx : cidx + 1],
                        )

            def emit_group_update(w, i, kpos, from_sign):
                stt = wstt[w]
                sl = slice(kpos * G, (kpos + 1) * G)
                tau = stt["tau"][:, sl]
                r = stt["r"][:, sl]
                c = stt["c"][:, sl]
                # r = sx - acc
                nc.vector.tensor_tensor(
                    r[:], stt["sx"][:, sl], stt["acc"][:, sl], Alu.subtract
                )
                if from_sign:
                    # c = (K - cacc)/2, then guard c >= 1
                    nc.vector.tensor_scalar(
                        stt["cacc"][:, sl], stt["cacc"][:, sl], -0.5,
                        float(K) * 0.5, Alu.mult, Alu.add,
                    )
                nc.vector.tensor_scalar_max(c[:], stt["cacc"][:, sl], 1.0)
                nc.vector.reciprocal(stt["rcp"][:, sl], c[:])
                # stp = (r - 1) * rcp
                nc.vector.scalar_tensor_tensor(
                    stt["stp"][:, sl], r[:], -1.0, stt["rcp"][:, sl],
                    Alu.add, Alu.mult,
                )
                # tau += lam * stp
                nc.vector.scalar_tensor_tensor(
                    tau[:], stt["stp"][:, sl], float(LAM[i]), tau[:],
                    Alu.mult, Alu.add,
                )
                if i == N_SLOTS - 1:
                    nc.vector.tensor_scalar(
                        stt["ntau"][:, sl], tau[:], -1.0, None, Alu.mult
                    )

            def emit_wave_slot(w, i):
                nw = len(waves[w])
                s_grp = i % W
                p_grp = (i + 1) % W
                # engine order: cross-engine counts issued first so ScalarE/
                # GPSIMD start while VectorE chews its own passes; groups with
                # cross-engine counts are updated last.
                order = [k for k in range(nw) if k not in (s_grp, p_grp)]
                if p_grp < nw and p_grp not in order and p_grp != s_grp:
                    order.append(p_grp)
                if s_grp < nw:
                    order.append(s_grp)
                if s_grp < nw:
                    emit_group_counts(w, i, s_grp, "S")
                if p_grp < nw and p_grp != s_grp:
                    emit_group_counts(w, i, p_grp, "P")
                for kpos in order:
                    emit_group_mins(w, i, kpos)
                    if kpos not in (s_grp, p_grp):
                        emit_group_counts(w, i, kpos, "V")
                    elif kpos == p_grp and kpos != s_grp:
                        # the pool group's leftover tile on VectorE
                        stt = wstt[w]
                        g = waves[w][kpos]
                        for j in range(POOL_CNT_TILES, G):
                            cidx = kpos * G + j
                            nc.vector.tensor_scalar(
                                scrC[:], xhs[g][:, j * K : (j + 1) * K],
                                stt["tau"][:, cidx : cidx + 1], None,
                                Alu.is_gt, Alu.add,
                                accum_out=stt["cacc"][:, cidx : cidx + 1],
                            )
                    emit_group_update(w, i, kpos, from_sign=(kpos == s_grp))

            def emit_wave_finals(w):
                stt = wstt[w]
                for kpos, g in enumerate(waves[w]):
                    rows = slice(g * G * P, (g + 1) * G * P)
                    o_dram = out_ext.ap()[rows, :].rearrange(
                        "(t p) k -> p t k", p=P
                    )
                    xb = xbs[g]
                    on_v = kpos in V_FINAL_KPOS
                    for j in range(G):
                        cidx = col(w, g, j)
                        xbj = xb[:, j * K : (j + 1) * K]
                        if on_v:
                            # in-place relu(x - tau) on VectorE (f32 2x)
                            nc.vector.tensor_scalar(
                                xbj, xbj, stt["tau"][:, cidx : cidx + 1], 0.0,
                                Alu.subtract, Alu.max,
                            )
                        else:
                            nc.scalar.activation(
                                xbj, xbj, Act.Relu,
                                bias=stt["ntau"][:, cidx : cidx + 1],
                            )
                    # stores ride the Pool SWDGE queue; loads use SP
                    nc.gpsimd.dma_start(
                        out=o_dram, in_=xb[:].rearrange("p (t k) -> p t k", t=G)
                    )

            emit_wave_loads(0)
            emit_wave_casts(0)
            if NW > 1:
                emit_wave_loads(1)
            for w in range(NW):
                if w + 2 < NW:
                    emit_wave_loads(w + 2)
                if w + 1 < NW:
                    alloc_wave_stats(w + 1)
                emit_wave_slot(w, 0)
                for i in range(1, N_SLOTS):
                    # spread next wave's casts between this wave's slots
                    if w + 1 < NW and i - 1 < len(waves[w + 1]):
                        emit_cast_group(w + 1, i - 1)
                    emit_wave_slot(w, i)
                if w + 1 < NW:
                    for kpos in range(N_SLOTS - 1, len(waves[w + 1])):
                        emit_cast_group(w + 1, kpos)
                emit_wave_finals(w)

    nc.compile()
    return nc


def _get_nc():
    global _NC_CACHE
    if _NC_CACHE is None:
        _NC_CACHE = _build_nc()
    return _NC_CACHE


def _effective_alpha(alpha):
    # the module's ClampMin/ClampMax pair, verbatim in numpy
    a = np.asarray(alpha, dtype=np.float32)
    a = np.maximum(np.minimum(a, 0.0) - 1.0, 0.0) + 1.0 + np.maximum(a, 0.0)
    a = np.minimum(np.maximum(a, 0.0) - 2.0, 0.0) + 2.0 + np.minimum(a, 0.0)
    return a.astype(np.float32)


def _entmax_bisect_numpy(x, a, n_iter=50):
    """Generic-alpha fallback replicating the reference bisection in f32.
    Never taken for alpha in [1,2] (the clamp maps those to exactly 2.0)."""
    f32 = np.float32
    X = x.reshape(-1, K).astype(np.float32)
    am1 = (np.broadcast_to(a.reshape(1, H), (B, H)).reshape(-1)[
        np.arange(X.shape[0]) // Q
    ].astype(np.float32) - f32(1.0))[:, None]
    Xs = (X * am1).astype(np.float32)

    def p(s):
        pos = s > 0
        return np.where(
            pos, np.power(np.where(pos, s, f32(1.0)), (f32(1.0) / am1)), f32(0.0)
        ).astype(np.float32)

    mx = Xs.max(axis=1, keepdims=True).astype(np.float32)
    tau_lo = (mx - f32(1.0)).astype(np.float32)
    tau_hi = (mx - np.power(f32(1.0 / K), am1)).astype(np.float32)
    f_lo = (p(Xs - tau_lo).sum(axis=1, dtype=np.float32)[:, None] - f32(1.0)).astype(
        np.float32
    )
    dm = (tau_hi - tau_lo).astype(np.float32)
    tau_m = tau_lo.copy()
    for _ in range(n_iter):
        dm = (dm * f32(0.5)).astype(np.float32)
        tau_m = (tau_lo + dm).astype(np.float32)
        f_m = (p(Xs - tau_m).sum(axis=1, dtype=np.float32)[:, None] - f32(1.0)).astype(
            np.float32
        )
        tau_lo = np.where(f_m * f_lo >= 0, tau_m, tau_lo).astype(np.float32)
    pm = p(Xs - tau_m)
    s = pm.sum(axis=1, dtype=np.float32).astype(np.float32)[:, None]
    return (pm / s).astype(np.float32).reshape(B, H, Q, K)


def kernel(**inputs) -> np.ndarray:
    from concourse.bass_utils import run_bass_kernel_spmd

    x = np.ascontiguousarray(np.asarray(inputs["x"], dtype=np.float32))
    alpha = np.asarray(inputs.get("alpha", np.full((1, H), 1.5, np.float32)))
    a_eff = _effective_alpha(alpha)
    if not np.all(a_eff == np.float32(2.0)):
        # out-of-distribution alpha (outside [1,2]): generic slow path
        return _entmax_bisect_numpy(x, a_eff)

    shards = x.reshape(N_CORES, ROWS_PER_CORE, K)
    in_maps = [{"x": shards[i]} for i in range(N_CORES)]

    nc = _get_nc()
    res = run_bass_kernel_spmd(nc, in_maps, core_ids=list(range(N_CORES)))
    out = np.stack([res.results[i]["out"] for i in range(N_CORES)])
    return out.reshape(B, H, Q, K)
